# revision 8
# baseline (speedup 1.0000x reference)
"""Linear attention (ELU+1 feature map) on 8 TRN2 NeuronCores — v10.

Algorithm (see v2): bf16 projections; ELU+1 split DVE/ACT/DVE; per-pair
kvT = vtok^T @ ktok and ksum = ktok^T @ ones2 single-shot into PSUM,
DVE-accumulated in f32 (PSUM accumulation groups interleaved within a bank
are broken on HW); pair AllReduce of kv/ksum overlapped with the q
projection; G_p = kv_p @ Wo_p^T folds kv into the output projection;
KS_p (half-masked free-broadcast ksum) gives the denominator already
broadcast to 128 partitions in ONE matmul; qs = qhat * (1/dnB) in place
(DVE reciprocal + Pool multiply); out = qs^T @ G accumulated over pairs.

Scheduling (v5):
  - ONE PSUM pool for the whole kernel. Opening a new pool makes every new
    tile wait on ALL users of the closed pool (release-boundary barrier,
    measured 5.8us at each phase edge); instead phase 1.5's qp reuses the
    kp/vp tag, G/dnB reuse the kvt tag, yp reuses the pp tag, so cross-
    phase waits are per-slot and land exactly on the pipelined tail.
  - ONE transient SBUF pool for the same reason.
  - kv/ksum matmuls of token-tile tt-1 are emitted after the projections
    of tt; kvt has 4 PSUM bufs so the g2 matmul never waits on the DVE
    aggregation adds.
  - input DMAs ride HWDGE queues (sync for wk, scalar for the rest) in
    consumption order -- gpsimd SWDGE DMAs would occupy the Pool engine.
  - phase-2 unpack runs on Pool during phase 1.5; chunk c+1's
    denominator chains interleave with chunk c's output projection.
"""

import sys
import numpy as np

for _p in ("/opt/trn_rl_repo", "/opt/pypackages"):
    if _p not in sys.path:
        sys.path.append(_p)

import concourse.bacc as bacc
import concourse.mybir as mybir
import concourse.tile as tile
from concourse import bass_utils

F32 = mybir.dt.float32
BF16 = mybir.dt.bfloat16
ACTF = mybir.ActivationFunctionType
Alu = mybir.AluOpType

N_CORES = 8
B, T, C = 4, 4096, 1024
H, D = 16, 64
S = B * T // N_CORES          # 2048 tokens per core
NP = 8                        # head pairs (128 channels each)
TT = S // 128                 # 16 token tiles per core
PSTR = 130                    # kv slot: 128 kvT cols + 2 ksum cols
XCH = 512                     # xs DMA token chunk
OUT_DT = BF16                 # device output dtype (host converts to f32)

_cache = {}


def _emit(nc, tc, KT, xt_d, wk_d, wv_d, wq_d, wo_d, out_d):
    Exp = ACTF.Exp

    with (
        tc.tile_pool(name="wkv", bufs=1) as wkv,
        tc.tile_pool(name="wqo", bufs=1) as wqo,
        tc.tile_pool(name="persist", bufs=1) as sb,
        tc.tile_pool(name="trans", bufs=1) as tr,
        tc.tile_pool(name="psum", bufs=1, space="PSUM") as ps,
        tc.tile_pool(name="dram", bufs=1, space="DRAM") as dram,
    ):
        # ---- input DMAs, in consumption order --------------------------
        # sync: wk (gates the very first matmuls); scalar: everything else
        wk_sb, wv_sb = [], []
        for ct in range(KT):
            w = wkv.tile([128, C], BF16, tag="wkv", bufs=2 * KT,
                         name=f"wk{ct}")
            nc.sync.dma_start(w[:], wk_d[ct * 128:(ct + 1) * 128, :])
            wk_sb.append(w)

        xsall = sb.tile([128, KT * S], BF16, tag="xs", name="xsall")
        xs_sb = [xsall[:, ct * S:(ct + 1) * S] for ct in range(KT)]
        xs3 = xsall.rearrange("p (c s) -> p c s", s=S)
        xt3 = xt_d.rearrange("(c p) s -> p c s", p=128)
        nc.scalar.dma_start(xs3[:, :, 0:256], xt3[:, :, 0:256])
        nc.scalar.dma_start(xs3[:, :, 256:XCH], xt3[:, :, 256:XCH])
        for ct in range(KT):
            w = wkv.tile([128, C], BF16, tag="wkv", bufs=2 * KT,
                         name=f"wv{ct}")
            nc.sync.dma_start(w[:], wv_d[ct * 128:(ct + 1) * 128, :])
            wv_sb.append(w)
        nc.sync.dma_start(xs3[:, :, XCH:S], xt3[:, :, XCH:S])
        wqall = wqo.tile([128, KT * C], BF16, tag="wq", name="wqall")
        wq_sb = [wqall[:, ct * C:(ct + 1) * C] for ct in range(KT)]
        nc.sync.dma_start(wqall.rearrange("p (c k) -> p c k", k=C),
                          wq_d.rearrange("(c p) k -> p c k", p=128))
        woall = wqo.tile([128, NP * C], BF16, tag="wo", name="woall")
        wo_sb = [woall[:, p * C:(p + 1) * C] for p in range(NP)]
        nc.sync.dma_start(woall.rearrange("p (c k) -> p c k", k=C),
                          wo_d.rearrange("(c p) k -> p c k",
                                         p=128)[:, 0:NP, :])

        ones2 = sb.tile([128, 2], BF16, tag="ones2", name="ones2")
        nc.gpsimd.memset(ones2[:], 1.0)
        onesks = sb.tile([128, 64], BF16, tag="onesks", name="onesks")
        nc.gpsimd.memset(onesks[:], 1.0)

        kvagg = sb.tile([128, NP * PSTR], F32, tag="kvagg", name="kvagg")
        nc.gpsimd.memset(kvagg[:], 0.0)

        qhat = [sb.tile([128, S], BF16, tag="qhat", bufs=NP, name=f"qhat{p}")
                for p in range(NP)]

        # ---- phase 1: k/v projections + kvT/ksum (kv one tile late) ----
        ktoks = [None] * TT
        vtoks = [None] * TT

        def emit_proj(tt):
            t0 = tt * 128
            xb = [xs_sb[ct][:, t0:t0 + 128] for ct in range(KT)]
            kp = ps.tile([128, C], F32, tag="pp", bufs=2, name=f"kp{tt}")
            for ct in range(KT):       # ct-major: arrival-paced at startup
                for ch in range(2):
                    nc.tensor.matmul(
                        kp[:, ch * 512:(ch + 1) * 512], xb[ct],
                        wk_sb[ct][:, ch * 512:(ch + 1) * 512],
                        start=(ct == 0), stop=(ct == KT - 1))
            vp = ps.tile([128, C], F32, tag="pp", bufs=2, name=f"vp{tt}")
            for ct in range(KT):
                for ch in range(2):
                    nc.tensor.matmul(
                        vp[:, ch * 512:(ch + 1) * 512], xb[ct],
                        wv_sb[ct][:, ch * 512:(ch + 1) * 512],
                        start=(ct == 0), stop=(ct == KT - 1))
            km = tr.tile([128, C], BF16, tag="km", bufs=2, name=f"km{tt}")
            ke = tr.tile([128, C], BF16, tag="ke", bufs=2, name=f"ke{tt}")
            ktok = tr.tile([128, C], BF16, tag="ktok", bufs=3,
                           name=f"ktok{tt}")
            HS = (slice(0, 512), slice(512, 1024))
            for h in HS:
                nc.vector.tensor_scalar_min(km[:, h], kp[:, h], 0.0)
            for h in HS:
                nc.scalar.activation(ke[:, h], km[:, h], Exp)
            for h in HS:
                nc.vector.scalar_tensor_tensor(ktok[:, h], kp[:, h], 0.0,
                                               ke[:, h], Alu.max, Alu.add)
            vtok = tr.tile([128, C], BF16, tag="vtok", bufs=3,
                           name=f"vtok{tt}")
            nc.scalar.copy(vtok[:], vp[:])
            ktoks[tt], vtoks[tt] = ktok, vtok

        def emit_kv(tt):
            ktok, vtok = ktoks[tt], vtoks[tt]
            for g in range(3):
                p0, p1n = 3 * g, min(3 * g + 3, NP)
                kvt = ps.tile([128, (p1n - p0) * PSTR], F32, tag="kvt",
                              bufs=4, name=f"kvt{tt}_{g}",
                              padded_shape=[128, 512])
                for p in range(p0, p1n):
                    j = p - p0
                    nc.tensor.matmul(
                        kvt[:, j * PSTR:j * PSTR + 128],
                        vtok[:, p * 128:(p + 1) * 128],
                        ktok[:, p * 128:(p + 1) * 128],
                        start=True, stop=True)
                    nc.tensor.matmul(
                        kvt[:, j * PSTR + 128:j * PSTR + 130],
                        ktok[:, p * 128:(p + 1) * 128],
                        ones2[:], start=True, stop=True)
                nc.vector.tensor_add(
                    kvagg[:, p0 * PSTR:p1n * PSTR],
                    kvagg[:, p0 * PSTR:p1n * PSTR], kvt[:])

        for tt in range(TT):
            emit_proj(tt)
            if tt > 0:
                emit_kv(tt - 1)
        emit_kv(TT - 1)

        # ---- pair AllReduce (overlaps phase 1.5) -----------------------
        bounce_in = dram.tile([128, NP * PSTR], F32, name="bounce_in")
        bounce_out = dram.tile([128, NP * PSTR], F32, name="bounce_out")
        nc.sync.dma_start(bounce_in[:], kvagg[:])
        nc.gpsimd.collective_compute(
            "AllReduce", Alu.add,
            ins=[bounce_in.opt()], outs=[bounce_out.opt()],
            replica_groups=[[2 * i, 2 * i + 1] for i in range(N_CORES // 2)])
        kvcoll = sb.tile([128, NP * PSTR], F32, tag="kvcoll", name="kvcoll")
        nc.sync.dma_start(kvcoll[:], bounce_out[:])

        # ---- unpack on Pool: runs during phase 1.5 ---------------------
        kvbs, KS = [], []
        for p in range(NP):
            c0 = p * PSTR
            kvb = sb.tile([128, 128], BF16, tag="kvb", bufs=NP,
                          name=f"kvb{p}")
            nc.gpsimd.memset(kvb[:], 0.0)
            nc.gpsimd.tensor_copy(kvb[0:64, 0:64],
                                  kvcoll[0:64, c0:c0 + 64])
            nc.gpsimd.tensor_copy(kvb[64:128, 64:128],
                                  kvcoll[64:128, c0 + 64:c0 + 128])
            kvbs.append(kvb)
            ks = sb.tile([128, 128], BF16, tag="KS", bufs=NP, name=f"KS{p}")
            nc.gpsimd.memset(ks[:], 0.0)
            nc.gpsimd.tensor_scalar_mul(
                ks[0:64, 0:64], onesks[0:64, :],
                kvcoll[0:64, c0 + 128:c0 + 129])
            nc.gpsimd.tensor_scalar_mul(
                ks[64:128, 64:128], onesks[64:128, :],
                kvcoll[64:128, c0 + 128:c0 + 129])
            KS.append(ks)

        # qs = qhat * 1/(KS^T qhat): denominator matmul, reciprocal,
        # in-place Pool multiply. Chunk-0 chains are pre-emitted inside the
        # phase-1.5 tail so the output projection starts immediately.
        def emit_scale(p, chk, mul_eng=None):
            qsl = qhat[p][:, chk * 512:(chk + 1) * 512]
            dnb = ps.tile([128, 512], F32, tag="kvt", bufs=4,
                          name=f"dnb{p}_{chk}")
            nc.tensor.matmul(dnb[:], KS[p][:], qsl, start=True, stop=True)
            rpb = tr.tile([128, 512], BF16, tag="rpb", bufs=3,
                          name=f"rpb{p}_{chk}")
            with nc.allow_low_precision(reason="recip of denom"):
                nc.vector.reciprocal(rpb[:], dnb[:])
            # Pool's 0.42-efficiency multiply is 1.1us; the last chunk-0
            # chains gate the first output group, so they ride DVE (0.66us)
            (mul_eng or nc.gpsimd).tensor_mul(qsl, qsl, rpb[:])

        G = [sb.tile([128, C], BF16, tag="G", bufs=NP, name=f"G{p}")
             for p in range(NP)]

        def emit_g(p):
            for ch in range(2):
                gp = ps.tile([128, 512], F32, tag="kvt", bufs=4,
                             name=f"gp{p}_{ch}")
                nc.tensor.matmul(gp[:], kvbs[p][:],
                                 wo_sb[p][:, ch * 512:(ch + 1) * 512],
                                 start=True, stop=True)
                nc.scalar.copy(G[p][:, ch * 512:(ch + 1) * 512], gp[:])

        # ---- phase 1.5: q projection + ELU, with the G build and the
        # chunk-0 denominator chains interleaved (their inputs are ready
        # once the AllReduce lands mid-phase) ------------------------------
        for p in range(NP):
            if p >= 4:
                emit_g(2 * (p - 4))
                emit_g(2 * (p - 4) + 1)
            if p == 3:
                emit_scale(0, 0)
                emit_scale(1, 0)
            elif p >= 4:
                emit_scale(p - 2, 0)
            for hh in range(2):
                if p == NP - 1 and hh == 1:
                    emit_scale(NP - 2, 0, mul_eng=nc.vector)
                h0 = hh * 1024
                qp = ps.tile([128, 1024], F32, tag="pp", bufs=2,
                             name=f"qp{p}_{hh}")
                for chk in range(2):
                    for ct in range(KT):
                        nc.tensor.matmul(
                            qp[:, chk * 512:(chk + 1) * 512],
                            wq_sb[ct][:, p * 128:(p + 1) * 128],
                            xs_sb[ct][:, h0 + chk * 512:
                                       h0 + (chk + 1) * 512],
                            start=(ct == 0), stop=(ct == KT - 1))
                qm = tr.tile([128, 1024], BF16, tag="qm", bufs=2,
                             name=f"qm{p}_{hh}")
                qe = tr.tile([128, 1024], BF16, tag="qe", bufs=2,
                             name=f"qe{p}_{hh}")
                HS = (slice(0, 512), slice(512, 1024))
                for hs in HS:
                    nc.vector.tensor_scalar_min(qm[:, hs], qp[:, hs], 0.0)
                for hs in HS:
                    nc.scalar.activation(qe[:, hs], qm[:, hs], Exp)
                for hs in HS:
                    nc.vector.scalar_tensor_tensor(
                        qhat[p][:, h0 + hs.start:h0 + hs.stop], qp[:, hs],
                        0.0, qe[:, hs], Alu.max, Alu.add)

        # ---- phase 2: remaining denominators + output projection -------

        emit_scale(NP - 1, 0, mul_eng=nc.vector)
        for chk in range(S // 512):
            groups = [(mt, ch) for mt in range(chk * 4, chk * 4 + 4)
                      for ch in range(2)]
            for i, (mt, ch) in enumerate(groups):
                if chk + 1 < S // 512 and i < NP:
                    emit_scale(i, chk + 1)
                r0 = mt * 128
                yp = ps.tile([128, 512], F32, tag="kvt", bufs=4,
                             name=f"yp{mt}_{ch}")
                for p in range(NP):
                    nc.tensor.matmul(
                        yp[:], qhat[p][:, r0:r0 + 128],
                        G[p][:, ch * 512:(ch + 1) * 512],
                        start=(p == 0), stop=(p == NP - 1))
                ysb = tr.tile([128, 512], BF16, tag="ysb", bufs=3,
                              name=f"ysb{mt}_{ch}")
                nc.scalar.copy(ysb[:], yp[:])
                nc.sync.dma_start(
                    out_d[r0:r0 + 128, ch * 512:(ch + 1) * 512], ysb[:])


def _build(has_bias: bool):
    KT = 9 if has_bias else 8
    KC = KT * 128

    nc = bacc.Bacc("TRN2", target_bir_lowering=False, debug=False,
                   num_devices=N_CORES)
    xt_d = nc.dram_tensor("xt", [KC, S], BF16, kind="ExternalInput").ap()
    wk_d = nc.dram_tensor("wkt", [KC, C], BF16, kind="ExternalInput").ap()
    wv_d = nc.dram_tensor("wvt", [KC, C], BF16, kind="ExternalInput").ap()
    wq_d = nc.dram_tensor("wqt", [KC, C], BF16, kind="ExternalInput").ap()
    wo_d = nc.dram_tensor("wot", [KC, C], BF16, kind="ExternalInput").ap()
    out_d = nc.dram_tensor("out", [S, C], BF16, kind="ExternalOutput").ap()

    with tile.TileContext(nc) as tc:
        _emit(nc, tc, KT, xt_d, wk_d, wv_d, wq_d, wo_d, out_d)
    nc.compile()
    return nc


def _prep_host(inputs, KT):
    """Host-side shard + transpose prep. Returns in_maps for the 8 cores."""
    KC = KT * 128
    npdt = mybir.dt.np(BF16)
    x = np.asarray(inputs["x"], np.float32).reshape(B * T, C)

    def padw(w, b):
        wt = np.ascontiguousarray(np.asarray(w, np.float32).T)  # [Cin, Cout]
        if KC == C:
            return wt.astype(npdt)
        out = np.zeros((KC, C), np.float32)
        out[:C] = wt
        out[C] = np.asarray(b, np.float32)
        return out.astype(npdt)

    wkt = padw(inputs["Wk"], inputs["bk"])
    wvt = padw(inputs["Wv"], inputs["bv"])
    wqt = padw(inputs["Wq"], inputs["bq"])
    wot = padw(inputs["Wo"], np.zeros(C))   # bo applied on host

    in_maps = []
    for c in range(N_CORES):
        sh = x[c * S:(c + 1) * S]
        xt = np.zeros((KC, S), np.float32)
        xt[:C] = sh.T
        if KC > C:
            xt[C] = 1.0
        in_maps.append({
            "xt": np.ascontiguousarray(xt.astype(npdt)),
            "wkt": wkt, "wvt": wvt, "wqt": wqt, "wot": wot,
        })
    return in_maps


def _get_nc(has_bias):
    if has_bias not in _cache:
        _cache[has_bias] = _build(has_bias)
    return _cache[has_bias]


def kernel(**inputs):
    assert np.asarray(inputs["x"]).shape == (B, T, C)
    has_bias = any(
        np.any(np.asarray(inputs[k])) for k in ("bq", "bk", "bv"))
    nc = _get_nc(has_bias)
    in_maps = _prep_host(inputs, 9 if has_bias else 8)
    res = bass_utils.run_bass_kernel_spmd(
        nc, in_maps, core_ids=list(range(N_CORES)))
    y = np.concatenate(
        [np.asarray(res.results[c]["out"], np.float32)
         for c in range(N_CORES)], axis=0)
    y = y.reshape(B, T, C)
    bo = np.asarray(inputs["bo"], np.float32)
    if np.any(bo):
        y = y + bo
    return y
